# revision 29
# baseline (speedup 1.0000x reference)
"""FFD sparse-matmul kernel for Trainium2 (8 NeuronCores).

Problem: out[b, r, d] = sum_i 1[rows_i == r] * vals_i * (x[b, cols_i, d]*scale[d] - offset[d])
  = (A @ (x*scale))[b, r, d] - rowsum[r] * offset[d]
where A is the static [200000, 4096] sparse FFD matrix (12.8M nnz).

Strategy: densify the static sparse matrix on the host, quantize per-row to
the fp8 e4m3 grid (1 byte/element), and stream it through the TensorEngine
as stationary weights against the tiny dynamic [4096, 6] fp16 control-point
matrix (batch*3 = 6 columns). fp8 weights keep the DMA path on plain HWDGE
(no SWDGE u8->f16 cast, which doubled SBUF-side bytes and saturated all 16
SDMA engines in the previous version). Mixed fp8 stationary x fp16 moving
matmul accumulates exactly in fp32 PSUM; a per-row dequant scale and the
affine-offset bias fold into one DVE multiply + subtract per row-block.
Row-sharded over 8 cores (25000 rows each).
"""

import os
import numpy as np
import ml_dtypes

N_PTS = 200000
N_CTRL = 4096
B = 2
N_CORES = 8
ROWS_PER_CORE = N_PTS // N_CORES  # 25000
BLK = 128
FN = B * 3  # 6 output columns (batch-major: j = b*3 + d)
GROUP = int(os.environ.get("K_GROUP", "4"))  # row-blocks per weight DMA
WBUFS = int(os.environ.get("K_WBUFS", "6"))  # weight-group SBUF buffers
NRINGS = int(os.environ.get("K_NRINGS", "2"))  # DMA queues carrying weights
F8_MAX = 240.0  # conservative e4m3 top-of-scale (IEEE e4m3 max)

LAST_RESULTS = None  # BassKernelResults of the most recent device run

_static_cache = {}  # fingerprint -> (wT_per_core, rowsum_pad, rscale, residsum)
_nc_cache = {}


def _fingerprint(*arrays):
    h = 0
    for a in arrays:
        s = a[:: max(1, a.size // 4096)].tobytes()
        h ^= hash((a.size, s, float(a.astype(np.float64).sum())))
    return h


def _install_profile_shim():
    """Make trace=True work in images whose antenv lacks axon_hooks, and
    neuter the bucket artifact upload. Best-effort; harmless if partial."""
    import sys
    import types

    try:
        import concourse.bass_utils as bu

        bu.upload_artifacts = lambda tmpdir: f"local:{tmpdir}"
    except Exception:
        pass
    try:
        import antenv.axon_hooks  # noqa: F401

        return
    except ImportError:
        pass
    try:
        mod = types.ModuleType("antenv.axon_hooks")
        mod._hook = None
        mod.set_axon_ntff_profile_hook = lambda h: setattr(mod, "_hook", h)
        mod.get_axon_ntff_profile_hook = lambda: mod._hook
        sys.modules["antenv.axon_hooks"] = mod
        import antenv

        antenv.axon_hooks = mod
        if "/root/.axon_site/trn_agent_boot" not in sys.path:
            sys.path.insert(0, "/root/.axon_site/trn_agent_boot")
        from trn_boot import _ntff_profile_via_ctypes

        hook = _ntff_profile_via_ctypes("/opt/axon/libaxon_pjrt.so")
        if hook is not None:
            mod._hook = hook
    except Exception:
        pass


def _build_nc(n_blocks, n_chunks, group):
    import concourse.mybir as mybir
    from concourse import bacc
    from concourse.tile import TileContext

    assert n_blocks % group == 0
    f8, f16, f32 = mybir.dt.float8e4, mybir.dt.float16, mybir.dt.float32
    nc = bacc.Bacc()
    n_groups = n_blocks // group
    wT = nc.declare_dram_parameter(
        "wT", [n_groups, BLK, group * n_chunks * BLK], f8, isOutput=False
    )
    xs = nc.declare_dram_parameter("xs", [BLK, n_chunks * FN], f16, isOutput=False)
    bias = nc.declare_dram_parameter("bias", [BLK, n_blocks * FN], f16, isOutput=False)
    rsc = nc.declare_dram_parameter("rscale", [BLK, n_blocks], f32, isOutput=False)
    # Output staged and stored as f16: halves the output write traffic and
    # the tail store; adds ~1e-3 relative error (well within the gate).
    out = nc.declare_dram_parameter("out", [BLK, n_blocks * FN], f16, isOutput=True)

    with TileContext(nc) as tc:
        with (
            tc.tile_pool(name="wp", bufs=WBUFS) as wp,
            tc.tile_pool(name="cp", bufs=1) as cp,
            tc.tile_pool(name="pp", bufs=4, space="PSUM") as pp,
        ):
            # Issue the first weight-group DMA before anything else so the
            # weight stream starts as early as possible. Weight groups
            # alternate between the two HWDGE rings (sync/scalar) so packet
            # streams of consecutive groups interleave across the engines.
            # Group 0 lands block-by-block so the PE starts on block 0 after
            # ~0.5 MB instead of waiting for the full group.
            gcols = group * n_chunks * BLK
            bcols = n_chunks * BLK
            w_first = wp.tile([BLK, gcols], f8, tag="w")
            x_sb = cp.tile([BLK, n_chunks * FN], f16, tag="x")
            nc.scalar.dma_start(out=x_sb[:], in_=xs[:])
            for g in range(group):
                nc.sync.dma_start(
                    out=w_first[:, g * bcols : (g + 1) * bcols],
                    in_=wT[0, :, g * bcols : (g + 1) * bcols],
                )
            bias_sb = cp.tile([BLK, n_blocks * FN], f16, tag="bias")
            nc.scalar.dma_start(out=bias_sb[:], in_=bias[:])
            rsc_sb = cp.tile([BLK, n_blocks], f32, tag="rsc")
            nc.scalar.dma_start(out=rsc_sb[:], in_=rsc[:])
            # Output staging is split into one tile per store chunk so a
            # store DMA never WAR-blocks later DVE writes.
            marks = [0, n_groups - 1, n_groups]
            oranges = [
                (m0 * group, m1 * group) for m0, m1 in zip(marks[:-1], marks[1:])
            ]
            obufs = [
                cp.tile(
                    [BLK, (b1 - b0) * FN], f16, tag=f"obuf{i}", name=f"obuf{i}"
                )
                for i, (b0, b1) in enumerate(oranges)
            ]

            def oslice_of(blk):
                for t, (b0, b1) in zip(obufs, oranges):
                    if b0 <= blk < b1:
                        return t[:, (blk - b0) * FN : (blk - b0 + 1) * FN]
                raise AssertionError(blk)
            # Prime DVE on the const-DMA semaphores so per-block DVE ops only
            # need the PE wait (DVE TensorTensor allows one sync wait each).
            scratch = cp.tile([BLK, 1], f32, tag="scratch")
            nc.vector.tensor_tensor(
                out=scratch[:],
                in0=bias_sb[:, :1],
                in1=bias_sb[:, :1],
                op=mybir.AluOpType.add,
            )
            nc.vector.tensor_tensor(
                out=scratch[:],
                in0=rsc_sb[:, :1],
                in1=rsc_sb[:, :1],
                op=mybir.AluOpType.add,
            )
            # Store finished output chunks mid-stream so only the last
            # group's slice remains for the kernel tail.
            store_at = {m1: i for i, m1 in enumerate(marks[1:-1])}  # gi -> chunk
            for gi in range(n_groups):
                if gi == 0:
                    w_sb = w_first
                else:
                    w_sb = wp.tile([BLK, gcols], f8, tag="w")
                    # Round-robin the weight stream over up to three
                    # descriptor paths (2x HWDGE + SWDGE): a single ring
                    # serializes per-dma_start completion latency (~1.5us),
                    # which starves the SDMA engines between groups.
                    w_eng = [nc.sync, nc.scalar, nc.gpsimd][gi % NRINGS]
                    w_eng.dma_start(out=w_sb[:], in_=wT[gi])
                if gi in store_at:
                    i = store_at[gi]
                    b0, b1 = oranges[i]
                    nc.sync.dma_start(
                        out=out[:, b0 * FN : b1 * FN], in_=obufs[i][:]
                    )
                for g in range(group):
                    blk = gi * group + g
                    ps = pp.tile([BLK, FN], f32)
                    for kc in range(n_chunks):
                        off = g * n_chunks * BLK + kc * BLK
                        nc.tensor.matmul(
                            ps[:],
                            w_sb[:, off : off + BLK],
                            x_sb[:, kc * FN : (kc + 1) * FN],
                            start=(kc == 0),
                            stop=(kc == n_chunks - 1),
                        )
                    oslice = oslice_of(blk)
                    # out = ps * rscale_r - bias
                    nc.vector.tensor_tensor(
                        out=oslice,
                        in0=ps[:],
                        in1=rsc_sb[:, blk : blk + 1].to_broadcast([BLK, FN]),
                        op=mybir.AluOpType.mult,
                    )
                    nc.vector.tensor_tensor(
                        out=oslice,
                        in0=oslice,
                        in1=bias_sb[:, blk * FN : (blk + 1) * FN],
                        op=mybir.AluOpType.subtract,
                    )
            b0, b1 = oranges[-1]
            nc.scalar.dma_start(out=out[:, b0 * FN :], in_=obufs[-1][:])
    nc.finalize()
    return nc


def _prepare_static(ffd_vals, ffd_rows, ffd_cols):
    """Densify the static sparse matrix into per-core fp8 weight tiles."""
    key = _fingerprint(ffd_vals, ffd_rows, ffd_cols)
    if key in _static_cache:
        return _static_cache[key]

    n_blocks = -(-ROWS_PER_CORE // BLK)  # 196
    r_pad = n_blocks * BLK  # 25088
    n_chunks = N_CTRL // BLK  # 32

    rowsum = np.bincount(
        ffd_rows, weights=ffd_vals.astype(np.float64), minlength=N_PTS
    ).astype(np.float32)

    try:
        from scipy.sparse import coo_matrix

        A = np.asarray(
            coo_matrix(
                (ffd_vals, (ffd_rows, ffd_cols)), shape=(N_PTS, N_CTRL)
            ).todense(),
            dtype=np.float32,
        )
    except Exception:
        A = np.zeros((N_PTS, N_CTRL), np.float32)
        np.add.at(A, (ffd_rows, ffd_cols), ffd_vals)

    wTs, rsums, rscales, rresids = [], [], [], []
    for c in range(N_CORES):
        Ac = A[c * ROWS_PER_CORE : (c + 1) * ROWS_PER_CORE]
        # Per-row scaling onto the e4m3 grid: Q = e4m3(A/s), s = rowmax/240.
        # Zeros stay exactly zero; error is ~5-bit (value-relative 1/32).
        rowmax = np.maximum(Ac.max(axis=1), 1e-30).astype(np.float32)
        s = rowmax / F8_MAX
        Q = (Ac / s[:, None]).astype(ml_dtypes.float8_e4m3)
        # Residual row-sums fold the mean quantization error into the bias:
        # A@xs ~= s*(Q@xs) + rowresid*mean(xs) with rowresid = sum_c (A - s*Q).
        resid = Ac.astype(np.float64) - (
            Q.astype(np.float64) * s[:, None].astype(np.float64)
        )
        rresid = resid.sum(axis=1).astype(np.float32)
        Ap = np.zeros((r_pad, N_CTRL), ml_dtypes.float8_e4m3)
        Ap[:ROWS_PER_CORE] = Q
        s_pad = np.ones(r_pad, np.float32)
        s_pad[:ROWS_PER_CORE] = s
        rr_pad = np.zeros(r_pad, np.float32)
        rr_pad[:ROWS_PER_CORE] = rresid
        # rscale[p, blk] = s_pad[blk*BLK + p]
        rscales.append(np.ascontiguousarray(s_pad.reshape(n_blocks, BLK).T))
        rresids.append(rr_pad)
        # wT[blk, p, kc*BLK + m] = Ap[blk*BLK + m, kc*BLK + p], then group
        # GROUP consecutive blocks into one contiguous [BLK, GROUP*4096] row.
        w = Ap.reshape(n_blocks, BLK, n_chunks, BLK)  # [blk, m, kc, p]
        w = w.transpose(0, 3, 2, 1).reshape(n_blocks, BLK, n_chunks * BLK)
        w = np.ascontiguousarray(
            w.reshape(n_blocks // GROUP, GROUP, BLK, n_chunks * BLK).transpose(
                0, 2, 1, 3
            )
        )
        wTs.append(w.reshape(n_blocks // GROUP, BLK, GROUP * n_chunks * BLK))
        rs = np.zeros(r_pad, np.float32)
        rs[:ROWS_PER_CORE] = rowsum[c * ROWS_PER_CORE : (c + 1) * ROWS_PER_CORE]
        rsums.append(rs)

    _static_cache.clear()
    _static_cache[key] = (wTs, rsums, rscales, rresids)
    return wTs, rsums, rscales, rresids


def kernel(x, scale_vec, offset, ffd_vals, ffd_rows, ffd_cols):
    global LAST_RESULTS
    from concourse.bass_utils import run_bass_kernel_spmd

    x = np.asarray(x, np.float32)
    scale_vec = np.asarray(scale_vec, np.float32)
    offset = np.asarray(offset, np.float32)
    ffd_vals = np.asarray(ffd_vals, np.float32)
    ffd_rows = np.asarray(ffd_rows, np.int32)
    ffd_cols = np.asarray(ffd_cols, np.int32)

    n_blocks = -(-ROWS_PER_CORE // BLK)
    r_pad = n_blocks * BLK
    n_chunks = N_CTRL // BLK

    wTs, rsums, rscales, rresids = _prepare_static(ffd_vals, ffd_rows, ffd_cols)

    # Dynamic (per-call) host prep: tiny.
    x6 = (x * scale_vec[None, None, :]).transpose(1, 0, 2).reshape(N_CTRL, FN)
    xmean = x6.mean(axis=0).astype(np.float32)  # for residual bias correction
    x6 = x6.astype(np.float16)
    # Device layout: x_tiled[k, kc*FN + j] = x6[kc*BLK + k, j]
    x_tiled = np.ascontiguousarray(
        x6.reshape(n_chunks, BLK, FN).transpose(1, 0, 2).reshape(BLK, n_chunks * FN)
    )
    off6 = np.tile(offset, B).astype(np.float32)  # j = b*3 + d -> offset[d]

    in_maps = []
    for c in range(N_CORES):
        # bias = rowsum*offset - rowresid*mean(xs)  (both fold per row, per j)
        bias6 = (
            rsums[c][:, None] * off6[None, :]
            - rresids[c][:, None] * xmean[None, :]
        )  # [r_pad, FN] fp32
        # Device layout: bias[p, blk*FN + j] = bias6[blk*BLK + p, j]
        bias_t = np.ascontiguousarray(
            bias6.reshape(n_blocks, BLK, FN)
            .transpose(1, 0, 2)
            .reshape(BLK, n_blocks * FN)
        ).astype(np.float16)
        in_maps.append(
            {"wT": wTs[c], "xs": x_tiled, "bias": bias_t, "rscale": rscales[c]}
        )

    if ("nc", n_blocks, "f8") not in _nc_cache:
        _nc_cache[("nc", n_blocks, "f8")] = _build_nc(n_blocks, n_chunks, GROUP)
    nc = _nc_cache[("nc", n_blocks, "f8")]

    trace = bool(os.environ.get("BASS_TRACE"))
    if trace:
        _install_profile_shim()
    try:
        res = run_bass_kernel_spmd(nc, in_maps, list(range(N_CORES)), trace=trace)
    except Exception:
        if not trace:
            raise
        os.environ.pop("BASS_TRACE", None)
        res = run_bass_kernel_spmd(nc, in_maps, list(range(N_CORES)), trace=False)
    LAST_RESULTS = res

    shards = []
    for c in range(N_CORES):
        o = np.asarray(res.results[c]["out"], np.float32)  # [BLK, n_blocks*FN]
        o6 = (
            o.reshape(BLK, n_blocks, FN)
            .transpose(1, 0, 2)
            .reshape(r_pad, FN)[:ROWS_PER_CORE]
        )
        shards.append(o6)
    full6 = np.concatenate(shards, axis=0)  # [N_PTS, FN]
    out = np.ascontiguousarray(
        full6.reshape(N_PTS, B, 3).transpose(1, 0, 2)
    ).astype(np.float32)
    return out
